# revision 1
# baseline (speedup 1.0000x reference)
"""Sliding-window causal attention (B=2, H=16, T=2048, D=64, WINDOW=512) on
8 TRN2 NeuronCores.

Sharding: the 32 (b, h) pairs are split 4-per-core (embarrassingly parallel).
Each core runs the same Bass/Tile program over its 4 heads (2 pairs).

Structure (vs the original baseline, 95.7us -> 88.0us measured):
  - exp on the ACT engine is the steady-state bottleneck (64 tiles x
    ~0.73us); the whole pipeline is organized to keep ACT dense.
  - QK matmuls of the two heads of a pair are emitted zipped chunk-by-chunk;
    head A on partitions 0:64, head B on 64:128 auto-derive tile_position
    (0,0)/(64,0), and when both are ready they co-execute in different
    row-groups of the PE array (observed dt ~3ns).  s_ps bufs=3 decouples
    QK(kb+1) from exp(kb) so both heads are usually ready together.
  - A 30-matmul dummy warmup burst on a scratch PSUM region flips the PE
    HAM clock gate from 1.2 to 2.4 GHz while the first DMAs land.
  - PV accumulates 2 query blocks per head into one shared PSUM bank-tile;
    normalize (reciprocal + broadcast mul) reads PSUM directly -- no
    per-block drain copies.
  - Masking is one strided bf16 tensor_mul per block on DVE (const 0/1
    triangle masks [128,2,128]) instead of gpsimd affine_selects.
  - Input staging is DMA-rate limited (256B packets): the three units that
    gate QK(0) are split across the sync+scalar HW-DGE queues, V rides the
    gpsimd SWDGE queue, later units trickle through the attention loop, and
    pair 1 is staged inside pair 0's attention so ACT never drains at the
    pair boundary.  The final output flush is split sync/scalar to halve
    the tail.
"""

import sys
from contextlib import ExitStack

import numpy as np

sys.path.insert(0, "/opt/trn_rl_repo")

import concourse.bacc as bacc
import concourse.tile as tile
from concourse import mybir
from concourse.bass_utils import run_bass_kernel_spmd

F32 = mybir.dt.float32
BF16 = mybir.dt.bfloat16
EXP = mybir.ActivationFunctionType.Exp

B, H, T, D = 2, 16, 2048, 64
WINDOW = 512
SCALE = D ** -0.5
N_CORES = 8
HEADS_PER_CORE = (B * H) // N_CORES  # 4
TB = T // 128  # 16 query/key blocks
TH = 1024  # half-sequence tile width for qd/kd


def build_nc(t=T, heads_per_core=HEADS_PER_CORE):
    nb = t // 128

    nc = bacc.Bacc("TRN2", target_bir_lowering=False)
    q_ext = nc.declare_dram_parameter("q", [heads_per_core, t, D], F32, isOutput=False)
    k_ext = nc.declare_dram_parameter("k", [heads_per_core, t, D], F32, isOutput=False)
    v_ext = nc.declare_dram_parameter("v", [heads_per_core, t, D], F32, isOutput=False)
    id_ext = nc.declare_dram_parameter("ident", [128, 128], F32, isOutput=False)
    o_ext = nc.declare_dram_parameter("out", [heads_per_core, t, D], F32, isOutput=True)

    assert heads_per_core % 2 == 0
    n_pairs = heads_per_core // 2

    with tile.TileContext(nc) as tc, ExitStack() as ctx:
        const = ctx.enter_context(tc.tile_pool(name="const", bufs=1))
        stage = ctx.enter_context(tc.tile_pool(name="stage", bufs=6))
        vstage = ctx.enter_context(tc.tile_pool(name="vstage", bufs=2))
        qkd = ctx.enter_context(tc.tile_pool(name="qkd", bufs=2))
        vps = ctx.enter_context(tc.tile_pool(name="vps", bufs=4))
        ets = ctx.enter_context(tc.tile_pool(name="ets", bufs=13))
        outs = ctx.enter_context(tc.tile_pool(name="outs", bufs=3))
        rcp = ctx.enter_context(tc.tile_pool(name="rcp", bufs=4))
        # PSUM banks: 1 (trp) + 3*2 (sp) + 1 (shared ob/warmup) = 8
        tr_ps = ctx.enter_context(tc.tile_pool(name="tr_ps", bufs=1, space="PSUM"))
        s_ps = ctx.enter_context(tc.tile_pool(name="s_ps", bufs=3, space="PSUM"))
        ob_ps = ctx.enter_context(tc.tile_pool(name="ob_ps", bufs=1, space="PSUM"))

        # HAM warmup: the PE clock gate only opens (1.2 -> 2.4 GHz) after
        # ~3.4us of sustained matmul activity, and every real matmul here
        # runs ~2x faster warm.  Burn a dense burst of dummy matmuls on a
        # scratch PSUM bank while the first DMAs are in flight, and keep
        # feeding short dummy bursts between real matmul groups so the
        # activity monitor never re-throttles.
        dm_src = const.tile([128, 128], BF16, tag="dm_src")
        nc.vector.memset(dm_src[:], 0.0)
        dm_out = ob_ps.tile([128, 512], F32, tag="ob", name="ob_warm")

        def pe_dummy(n):
            for i in range(n):
                nc.tensor.matmul(
                    dm_out[:, 384:512], dm_src[:], dm_src[:], start=True, stop=True
                )



        # fp32 identity + bf16 copy (for Q/K transposes).
        ident_f = const.tile([128, 128], F32, tag="ident_f")
        nc.sync.dma_start(out=ident_f[:], in_=id_ext[:])
        ident_b = const.tile([128, 128], BF16, tag="ident_b")
        nc.vector.tensor_copy(ident_b[:], ident_f[:])

        # multiplicative 0/1 masks for the two boundary subtiles of E^T,
        # packed [128, 2, 128] so one strided tensor_mul masks both:
        # slot 0 keeps c >= r (causal diagonal), slot 1 keeps c < r (window).
        maskDW = const.tile([128, 2, 128], BF16, tag="maskDW")
        nc.gpsimd.memset(maskDW[:], 1.0)
        nc.gpsimd.affine_select(
            out=maskDW[:, 0, :], in_=maskDW[:, 0, :],
            compare_op=mybir.AluOpType.is_ge,
            fill=0.0, base=0, pattern=[[1, 128]], channel_multiplier=-1,
        )
        nc.gpsimd.affine_select(
            out=maskDW[:, 1, :], in_=maskDW[:, 1, :],
            compare_op=mybir.AluOpType.is_ge,
            fill=0.0, base=-1, pattern=[[-1, 128]], channel_multiplier=1,
        )

        # per-pair state
        qd_halves = {}
        kd_halves = {}
        vp = {}

        def alloc_pair(pair):
            qd_halves[pair] = [
                qkd.tile([128, TH], BF16, tag="qd0", name=f"qd0_{pair}"),
                qkd.tile([128, TH], BF16, tag="qd1", name=f"qd1_{pair}"),
            ]
            kd_halves[pair] = [
                qkd.tile([128, TH], BF16, tag="kd0", name=f"kd0_{pair}"),
                qkd.tile([128, TH], BF16, tag="kd1", name=f"kd1_{pair}"),
            ]

        def stage_dma(pair, ext, u, engs):
            # DMA one 512-row chunk of q or k (both heads) into a staging
            # tile; issue the two half-DMAs on separate engine queues.
            rows = slice(u * 512, (u + 1) * 512)
            st_f = stage.tile([128, 512], F32, tag="st_f")
            st3 = st_f[:].rearrange("p (b c) -> p b c", c=128)
            for eng, (hh, doff) in zip(
                engs, ((2 * pair, 0), (2 * pair + 1, 64))
            ):
                eng.dma_start(
                    out=st3[:, :, doff : doff + 64],
                    in_=ext[hh, rows, :].rearrange("(b p) d -> p b d", p=128),
                )
            return st_f

        def stage_compute(st_f, halves, u, cast_on_act=False):
            # cast -> 4 PE transposes -> drain into the d-major half
            st_b = stage.tile([128, 512], BF16, tag="st_b")
            if cast_on_act:
                # ACT queue is idle during the bootstrap; casting there
                # breaks the serial DVE cast chain
                nc.scalar.activation(
                    st_b[:], st_f[:], mybir.ActivationFunctionType.Copy
                )
            else:
                nc.vector.tensor_copy(st_b[:], st_f[:])
            trp = tr_ps.tile([128, 512], BF16, tag="trp")
            for i in range(4):
                nc.tensor.transpose(
                    trp[:, i * 128 : (i + 1) * 128],
                    st_b[:, i * 128 : (i + 1) * 128],
                    ident_b[:],
                )
            dst = halves[u // 2]
            dcol = (u % 2) * 512
            nc.vector.tensor_copy(dst[:, dcol : dcol + 512], trp[:, 0:512])

        def stage_unit(pair, ext, halves, u, engs):
            stage_compute(stage_dma(pair, ext, u, engs), halves, u)

        def stage_v(h):
            vst = vstage.tile([128, 1024], F32, tag="vst")
            v3 = vst[:].rearrange("p (b d) -> p b d", d=64)
            nc.gpsimd.dma_start(
                out=v3, in_=v_ext[h].rearrange("(b p) d -> p b d", p=128)
            )
            vt = vps.tile([128, nb, 65], BF16, tag="vp", name=f"vp_{h}")
            nc.vector.tensor_copy(vt[:, :, 0:64], v3)
            nc.gpsimd.memset(vt[:, :, 64:65], 1.0)
            vp[h] = vt

        def stage_feed(pair, engs):
            # closures that stage pair `pair`, to be interleaved into the
            # previous pair's attention loop (or run immediately).
            alloc_pair(pair)
            units = []
            units.append(lambda: stage_unit(pair, q_ext, qd_halves[pair], 0, engs))
            units.append(lambda: stage_unit(pair, k_ext, kd_halves[pair], 0, engs))
            units.append(lambda: stage_v(2 * pair))
            units.append(lambda: stage_v(2 * pair + 1))
            units.append(lambda: stage_unit(pair, q_ext, qd_halves[pair], 1, engs))
            units.append(lambda: stage_unit(pair, k_ext, kd_halves[pair], 1, engs))
            for u in (2, 3):
                units.append(
                    lambda u=u: stage_unit(pair, q_ext, qd_halves[pair], u, engs)
                )
                units.append(
                    lambda u=u: stage_unit(pair, k_ext, kd_halves[pair], u, engs)
                )
            return units

        def attention(pair, feed):
            # feed: dict kb -> list of closures (next pair's staging)
            hA, hB = 2 * pair, 2 * pair + 1
            rows_of = {hA: slice(0, 64), hB: slice(64, 128)}
            qdh, kdh = qd_halves[pair], kd_halves[pair]
            et = {hA: {}, hB: {}}
            sp_t = {}
            oo_t = {}
            oo3 = {}

            def emit_qk(kb):
                a = kb * 128
                span = min(640, t - a)
                for h in (hA, hB):
                    sp_t[h] = s_ps.tile([128, 1024], F32, tag="sp", name=f"sp_{h}_{kb}")
                chunks = []
                for lo in (0, TH):
                    s0, s1 = max(a, lo), min(a + span, lo + TH)
                    while s0 < s1:
                        n = min(512 - (s0 - a) % 512, s1 - s0)
                        chunks.append((s0 - a, lo // TH, s0 - lo, n))
                        s0 += n
                kd_half = kdh[a // TH]
                kcol = a % TH
                for (c, half, qc, n) in chunks:
                    for h in (hA, hB):
                        r = rows_of[h]
                        nc.tensor.matmul(
                            sp_t[h][:, c : c + n],
                            kd_half[r, kcol : kcol + 128],
                            qdh[half][r, qc : qc + n],
                            start=True,
                            stop=True,
                        )
                for h in (hA, hB):
                    e = ets.tile([128, 640], BF16, tag="et", name=f"et_{h}_{kb}")
                    et[h][kb] = e
                    nc.scalar.activation(e[:, 0:span], sp_t[h][:, 0:span], EXP, scale=SCALE)
                    if span == 640:
                        # one strided op masks both boundary triangles
                        e3 = e[:].rearrange("p (a b) -> p a b", b=128)
                        nc.vector.tensor_mul(
                            e3[:, 0:5:4, :], e3[:, 0:5:4, :], maskDW[:]
                        )
                    else:
                        nc.vector.tensor_mul(
                            e[:, 0:128], e[:, 0:128], maskDW[:, 0, :]
                        )

            ob_t = {}

            def emit_pv(qb):
                g, j = qb // 4, qb % 4
                g2, j2 = qb // 2, qb % 2
                for h in (hA, hB):
                    if j == 0:
                        oo_t[h] = outs.tile([128, 256], F32, tag="oo", name=f"oo_{h}_{g}")
                        oo3[h] = oo_t[h][:].rearrange("p (b d) -> p b d", d=64)
                    if j2 == 0 and h == hA:
                        # both heads' 2-qb accumulators share one bank-tile
                        ob_t[0] = ob_ps.tile([128, 512], F32, tag="ob", name=f"ob_{pair}_{g2}")
                    hoff = 0 if h == hA else 130
                    ob = ob_t[0][:, hoff : hoff + 130].rearrange(
                        "p (b c) -> p b c", c=65
                    )
                    kb0 = max(0, qb - 4)
                    for kb in range(kb0, qb + 1):
                        nc.tensor.matmul(
                            ob[:, j2, :],
                            et[h][kb][:, (qb - kb) * 128 : (qb - kb) * 128 + 128],
                            vp[h][:, kb, :],
                            start=(kb == kb0),
                            stop=(kb == qb),
                        )
                    if qb >= 4:
                        del et[h][qb - 4]
                    if j2 == 1:
                        # normalize 2 query blocks straight out of PSUM
                        rc = rcp.tile([128, 2], F32, tag="rc")
                        nc.vector.reciprocal(rc[:], ob[:, :, 64])
                        nc.vector.tensor_mul(
                            oo3[h][:, j - 1 : j + 1, :],
                            ob[:, :, 0:64],
                            rc[:].rearrange("p (b c) -> p b c", c=1).broadcast_to(
                                [128, 2, 64]
                            ),
                        )
                    if j == 3 or qb == nb - 1:
                        o_dst = o_ext[h, g * 512 : g * 512 + 512, :].rearrange(
                            "(b p) d -> p b d", p=128
                        )
                        if pair == n_pairs - 1 and g == 3:
                            nc.sync.dma_start(out=o_dst[:, 0:2, :], in_=oo3[h][:, 0:2, :])
                            nc.scalar.dma_start(out=o_dst[:, 2:4, :], in_=oo3[h][:, 2:4, :])
                        else:
                            nc.sync.dma_start(out=o_dst, in_=oo3[h][:, 0:4, :])

            for kb in range(nb + 1):
                if kb < nb:
                    emit_qk(kb)
                if kb >= 1:
                    emit_pv(kb - 1)
                for fn in feed.get(kb, ()):
                    fn()

        # pair 0: QK(0) needs q rows 0:1024 (u0q,u1q) and k rows 0:512
        # (u0k) -- spread those three units across four hardware DGE
        # queues so their transfers run concurrently; scalar/vector/tensor
        # queues are idle until the attention pipeline spins up.  The
        # DMA-issue ops are emitted before the warmup burst so the PE
        # queue issues its u0k DMA first.
        alloc_pair(0)
        stf_q0 = stage_dma(0, q_ext, 0, (nc.sync, nc.scalar))
        stf_k0 = stage_dma(0, k_ext, 0, (nc.sync, nc.scalar))
        stf_q1 = stage_dma(0, q_ext, 1, (nc.sync, nc.scalar))
        stage_v(0)
        stage_v(1)
        pe_dummy(30)
        stage_compute(stf_q0, qd_halves[0], 0)
        stage_compute(stf_k0, kd_halves[0], 0, cast_on_act=True)
        stage_compute(stf_q1, qd_halves[0], 1)
        dm_sink = const.tile([128, 1], F32, tag="dm_sink")
        nc.vector.tensor_copy(dm_sink[:], dm_out[:, 384:385])
        eng0 = (nc.sync, nc.gpsimd)
        feed0 = {
            0: [
                lambda: stage_unit(0, k_ext, kd_halves[0], 1, eng0),
                lambda: stage_unit(0, q_ext, qd_halves[0], 2, eng0),
            ],
            1: [lambda: stage_unit(0, k_ext, kd_halves[0], 2, eng0)],
            2: [lambda: stage_unit(0, q_ext, qd_halves[0], 3, eng0)],
            3: [lambda: stage_unit(0, k_ext, kd_halves[0], 3, eng0)],
        }
        # pair 1 staged during pair 0's attention, starting at kb=6
        units1 = stage_feed(1, eng0)
        feed1_in_0 = {6 + i: [units1[i]] for i in range(len(units1))}
        feed0.update(feed1_in_0)

        attention(0, feed0)
        attention(1, {})


    nc.compile()
    return nc


_NC_CACHE = {}
TRACE = False
TRACE_DIR = None
LAST_RESULT = None


def _get_nc():
    key = (T, HEADS_PER_CORE)
    if key not in _NC_CACHE:
        _NC_CACHE[key] = build_nc()
    return _NC_CACHE[key]


def kernel(q, k, v):
    q = np.ascontiguousarray(np.asarray(q, dtype=np.float32))
    k = np.ascontiguousarray(np.asarray(k, dtype=np.float32))
    v = np.ascontiguousarray(np.asarray(v, dtype=np.float32))
    assert q.shape == (B, H, T, D)

    qf = q.reshape(B * H, T, D)
    kf = k.reshape(B * H, T, D)
    vf = v.reshape(B * H, T, D)
    ident = np.eye(128, dtype=np.float32)

    in_maps = []
    for c in range(N_CORES):
        s = slice(c * HEADS_PER_CORE, (c + 1) * HEADS_PER_CORE)
        in_maps.append(
            {
                "q": np.ascontiguousarray(qf[s]),
                "k": np.ascontiguousarray(kf[s]),
                "v": np.ascontiguousarray(vf[s]),
                "ident": ident,
            }
        )

    nc = _get_nc()
    global LAST_RESULT
    res = run_bass_kernel_spmd(
        nc, in_maps, list(range(N_CORES)), trace=TRACE, tmpdir=TRACE_DIR
    )
    LAST_RESULT = res
    out = np.concatenate([res.results[c]["out"] for c in range(N_CORES)], axis=0)
    return out.reshape(B, H, T, D).astype(np.float32)

